# revision 27
# baseline (speedup 1.0000x reference)
"""Self-contained Trainium2 (Bass/Tile) kernel for nn_CausalSTDiTBlock_80058190397994.

kernel(**inputs) takes the FULL unsharded inputs (x, y, t, tpe, sst, weights)
and returns the full (4, 4096, 1152) float32 output, running SPMD across 8
NeuronCores. Sharding: core = (batch, spatial-half); AdaLN modulation /
gates are folded into per-core host-prepped weights and biases.

v2: projection GEMMs run in fp8(e4m3) with DoubleRow perf mode.
  Weights are host-scaled by 128 (folded back out via the eviction
  activation's scale); activations are staged unscaled in fp8.
  Modes per projection: 'fp8' (both operands single fp8; K=9 slabs ->
  5 DR matmuls via a duplicated activation slab and a halved last
  weight slab), 'wcomp' (weights split hi+lo fp8 to cancel weight
  quantization error; 9 DR matmuls), or 'bf16'.
  hT is streamed through SBUF in two token-halves (fc1 evicts gelu-fp8
  straight to SBUF; fc2 contracts from SBUF) - no DRAM staging.
  Weight loads use flat per-partition descriptors.

Device design notes (carried from v1):
  Residual x_res is feature-major (C x 2048) bf16, SBUF-resident.
  K biases are dropped everywhere (softmax-invariant). V biases fold into
  the following projection bias. tpe is added during virtual-order staging.
  qT_*/kT_* feature-major bf16 in DRAM; v_* token-major bf16.
  Temporal tensors use virtual order v = g*128 + t*8+sig; causal+block mask
  applied per tile. Attention runs in waves of 4 heads.
"""
import sys
sys.path.insert(0, "/opt/trn_rl_repo")
import numpy as np
from contextlib import ExitStack

import concourse.bass as bass
import concourse.mybir as mybir
import concourse.tile as tile
from concourse import bacc
from concourse.masks import make_identity

P = 128
T, C, NH, HD, YL = 16, 1152, 16, 72, 120
S, SH = 256, 128
NTOK = T * SH
GRP, NGRP = 8, 16
NC_C = C // P            # 9
NTT = NTOK // P          # 16
bf16 = mybir.dt.bfloat16
f8 = mybir.dt.float8e4
f32 = mybir.dt.float32
AF = mybir.ActivationFunctionType
ALU = mybir.AluOpType
DR = mybir.MatmulPerfMode.DoubleRow
SCALE = float(HD) ** -0.5
NW = 4                   # heads per attention wave
WS = 128.0               # fp8 weight scale (folded out at eviction)
WSI = 1.0 / WS

# per-projection matmul mode: 'fp8' | 'wcomp' | 'bf16'
# Error budget (emulated solo rel-err contributions): qkv_s wcomp ~4.8e-3,
# qkv_t fp8 ~1.0e-2, cross fp8 ~3e-3; proj_s/t and MLP stay bf16 (their
# fp8/wcomp variants alone cost 1.1e-2..2.7e-2). Total ~1.2e-2 < 2e-2 gate.
MODES = dict(q_s='wcomp', k_s='wcomp', v_s='wcomp',
             qk_t='fp8', v_t='fp8',
             q_c='fp8', k_c='fp8', v_c='fp8',
             proj_s='bf16', proj_t='bf16', proj_c='fp8',
             fc1='bf16', fc2='bf16')

XSLAB = NC_C + 1         # x staging tensors carry a dup of slab 8 at slot 9


def n_wslots(mode, nk=NC_C):
    if mode == 'bf16':
        return nk
    if mode == 'fp8':
        return nk + (nk % 2)
    if mode == 'wcomp':
        return 2 * nk
    raise ValueError(mode)


def dr_sched(mode, nk=NC_C):
    """[(wslot0, kslab0), ...] DoubleRow pair schedule."""
    if mode == 'fp8':
        return [(2 * i, 2 * i) for i in range((nk + 1) // 2)]
    if mode == 'wcomp':
        if nk % 2:
            hi = [(2 * i, 2 * i) for i in range(nk // 2)]
            mid = [(nk - 1, nk - 1)]       # (h_last, l_last) x (x8, dup)
            lo = [(nk + 1 + 2 * i, 2 * i) for i in range(nk // 2)]
            return hi + mid + lo
        return ([(2 * i, 2 * i) for i in range(nk // 2)]
                + [(nk + 2 * i, 2 * i) for i in range(nk // 2)])
    raise ValueError(mode)


def build(replicate: int = 1):
    nc = bacc.Bacc(num_devices=8)
    dp = lambda name, shape, dt: nc.declare_dram_parameter(name, list(shape), dt, isOutput=False)

    def wdt(mode):
        return bf16 if mode == 'bf16' else f8

    xT_own = dp("xT_own", (C, NTOK), bf16)
    xT_prt = dp("xT_prt", (C, NTOK), bf16)
    yT8 = dp("yT8", (P, XSLAB * P), f8)   # y slabs padded to 128 cols (16B DR alignment)
    # m-major weights: [p, m, slot, j] flat; slot count per mode
    wq_s = dp("wq_s", (P, NC_C * n_wslots(MODES['q_s']) * P), wdt(MODES['q_s']))
    wk_s = dp("wk_s", (P, NC_C * n_wslots(MODES['k_s']) * P), wdt(MODES['k_s']))
    wproj_s = dp("wproj_s", (P, NC_C * n_wslots(MODES['proj_s']) * P), wdt(MODES['proj_s']))
    wqk_t = dp("wqk_t", (P, 2 * NC_C * n_wslots(MODES['qk_t']) * P), wdt(MODES['qk_t']))
    wproj_t = dp("wproj_t", (P, NC_C * n_wslots(MODES['proj_t']) * P), wdt(MODES['proj_t']))
    wq_c = dp("wq_c", (P, NC_C * n_wslots(MODES['q_c']) * P), wdt(MODES['q_c']))
    wk_c = dp("wk_c", (P, NC_C * n_wslots(MODES['k_c']) * P), wdt(MODES['k_c']))
    wproj_c = dp("wproj_c", (P, NC_C * n_wslots(MODES['proj_c']) * P), wdt(MODES['proj_c']))
    wfc1 = dp("wfc1", (P, 4 * NC_C * n_wslots(MODES['fc1']) * P), wdt(MODES['fc1']))
    wfc2 = dp("wfc2", (P, NC_C * n_wslots(MODES['fc2'], 4 * NC_C) * P), wdt(MODES['fc2']))
    # j-contiguous layouts for token-major (V-style) projections: [p, slot, j(C)]
    wv_s = dp("wv_s", (P, n_wslots(MODES['v_s']) * C), wdt(MODES['v_s']))
    wv_t = dp("wv_t", (P, n_wslots(MODES['v_t']) * C), wdt(MODES['v_t']))
    wv_c = dp("wv_c", (P, n_wslots(MODES['v_c']) * C), wdt(MODES['v_c']))
    bq_s = dp("bq_s", (C,), f32)
    bqk_t = dp("bqk_t", (2 * C,), f32)
    bq_c = dp("bq_c", (C,), f32)
    bproj_s = dp("bproj_s", (C,), f32)
    bproj_t = dp("bproj_t", (C,), f32)
    bproj_c = dp("bproj_c", (C,), f32)
    bfc1 = dp("bfc1", (4 * C,), f32)
    bfc2 = dp("bfc2", (C,), f32)
    tpev = dp("tpev", (P, NC_C * T), bf16)   # tpe feature-major [p, k, t]
    maskneg = dp("maskneg", (P, P), bf16)
    outT = nc.declare_dram_parameter("outT", [C, NTOK], f32, isOutput=True)

    def sc_of(mode):
        return 1.0 if mode == 'bf16' else WSI

    with ExitStack() as ctx:
        tc = ctx.enter_context(tile.TileContext(nc))
        dr = ctx.enter_context(tc.tile_pool(name="dr", bufs=1, space="DRAM"))
        qT_s = dr.tile([C, NTOK], bf16, tag="qT_s")
        kT_all = dr.tile([C, T * S], bf16, tag="kT_all")
        v_s = dr.tile([T * S, NH * (HD + 1)], bf16, tag="v_s")
        qT_t = dr.tile([C, NTOK], bf16, tag="qT_t")
        kT_t = dr.tile([C, NTOK], bf16, tag="kT_t")
        v_t = dr.tile([NTOK, NH * (HD + 1)], bf16, tag="v_t")
        qT_c = dr.tile([C, NTOK], bf16, tag="qT_c")
        kT_y = dr.tile([C, YL], bf16, tag="kT_y")
        v_y = dr.tile([YL, NH * (HD + 1)], bf16, tag="v_y")
        big = ctx.enter_context(tc.tile_pool(name="big", bufs=1))
        cons = ctx.enter_context(tc.tile_pool(name="cons", bufs=1))
        wp = ctx.enter_context(tc.tile_pool(name="wp", bufs=2))
        lnp = ctx.enter_context(tc.tile_pool(name="lnp", bufs=3))
        sp = ctx.enter_context(tc.tile_pool(name="sp", bufs=2))
        sqp = ctx.enter_context(tc.tile_pool(name="sqp", bufs=4))
        bcp = ctx.enter_context(tc.tile_pool(name="bcp", bufs=4))
        lrow = ctx.enter_context(tc.tile_pool(name="lrow", bufs=1))
        ap_ = ctx.enter_context(tc.tile_pool(name="ap", bufs=2))
        fr = ctx.enter_context(tc.tile_pool(name="fr", bufs=2))
        pp = ctx.enter_context(tc.tile_pool(name="pp", bufs=2, space="PSUM"))
        psc = ctx.enter_context(tc.tile_pool(name="psc", bufs=4, space="PSUM"))
        pov = ctx.enter_context(tc.tile_pool(name="pov", bufs=2, space="PSUM"))

        # ---------- constants ----------
        ident = cons.tile([P, P], bf16, tag="ident")
        make_identity(nc, ident[:])
        ones16 = cons.tile([P, 16], bf16, tag="ones16")
        nc.vector.memset(ones16[:], 1.0)
        mask4 = cons.tile([P, NW, P], bf16, tag="mask4")
        for i in range(NW):
            nc.sync.dma_start(out=mask4[:, i, :], in_=maskneg[:, :])
        tpev_sb = cons.tile([P, NC_C, T], bf16, tag="tpev")
        nc.sync.dma_start(out=tpev_sb[:], in_=tpev.rearrange("p (k t) -> p k t", t=T))
        eps_t = cons.tile([1, 1], f32, tag="eps")
        nc.vector.memset(eps_t[:], 1e-6)

        def bias_cols(src, n, tag):
            t_ = cons.tile([P, n // P], f32, tag=tag)
            nc.gpsimd.dma_start(out=t_[:], in_=src.rearrange("(j p) -> p j", p=P))
            return t_
        b_q_s = bias_cols(bq_s, C, "b_q_s")
        b_qk_t = bias_cols(bqk_t, 2 * C, "b_qk_t")
        b_q_c = bias_cols(bq_c, C, "b_q_c")
        b_proj_s = bias_cols(bproj_s, C, "b_proj_s")
        b_proj_t = bias_cols(bproj_t, C, "b_proj_t")
        b_proj_c = bias_cols(bproj_c, C, "b_proj_c")
        b_fc1 = bias_cols(bfc1, 4 * C, "b_fc1")
        b_fc2 = bias_cols(bfc2, C, "b_fc2")

        x_res = big.tile([P, NC_C, NTOK], bf16, tag="xres")
        k_all_r = kT_all.rearrange("c (t s) -> c t s", t=T)

        def layer_norm(src_get, dst):
            """src_get(i, ch) -> bf16 AP (128 x 512); dst [P,XSLAB,NTOK] fp8.
            Two passes: stats+broadcasts for all chunks first, then applies.
            Slab 8 is duplicated into slot 9 for DoubleRow pairing."""
            bcs = []
            for ch in range(NTOK // 512):
                ps1 = pp.tile([1, 512], f32, tag="ps")
                ps2 = pp.tile([1, 512], f32, tag="ps")
                for i in range(NC_C):
                    xs = src_get(i, ch)
                    sq = sqp.tile([P, 512], bf16, tag="ln_sq")
                    if i % 2:
                        nc.scalar.activation(sq[:], xs, AF.Square)
                    else:
                        nc.vector.tensor_mul(sq[:], xs, xs)
                    nc.tensor.matmul(ps1[:], ones16[:, 0:1], xs,
                                     start=(i == 0), stop=(i == NC_C - 1))
                    nc.tensor.matmul(ps2[:], ones16[:, 0:1], sq[:],
                                     start=(i == 0), stop=(i == NC_C - 1))
                ra = lrow.tile([1, 512], f32, tag="ln_a")
                rb = lrow.tile([1, 512], f32, tag="ln_b")
                rc = lrow.tile([1, 512], f32, tag="ln_c")
                rd = lrow.tile([1, 512], bf16, tag="ln_d")
                nc.vector.tensor_scalar_mul(out=ra[:], in0=ps1[:], scalar1=1.0 / C)  # mu
                nc.vector.tensor_mul(rc[:], ra[:], ra[:])                            # mu^2
                nc.vector.scalar_tensor_tensor(out=rb[:], in0=ps2[:], scalar=1.0 / C,
                                               in1=rc[:], op0=ALU.mult,
                                               op1=ALU.subtract)                     # var
                nc.scalar.activation(rb[:], rb[:], AF.Sqrt, bias=eps_t[:])           # sd
                nc.vector.reciprocal(rc[:], rb[:])                                   # r
                nc.vector.tensor_mul(rd[:], ra[:], rc[:])                            # mu*r (bf16)
                rce = lrow.tile([1, 512], bf16, tag="ln_e")
                nc.vector.tensor_copy(out=rce[:], in_=rc[:])
                rbc = bcp.tile([P, 512], bf16, tag="ln_rbc")
                nc.gpsimd.partition_broadcast(rbc[:], rce[:])
                mbc = bcp.tile([P, 512], bf16, tag="ln_mbc")
                nc.gpsimd.partition_broadcast(mbc[:], rd[:])
                bcs.append((rbc, mbc))
            fp8_dst = dst.dtype == f8
            for ch, (rbc, mbc) in enumerate(bcs):
                for i in range(NC_C):
                    d = dst[:, i, ch * 512:(ch + 1) * 512]
                    eng = nc.gpsimd if i % 3 == 2 else nc.vector
                    if fp8_dst:
                        tmp = sqp.tile([P, 512], bf16, tag="ln_tmp", bufs=2)
                        eng.tensor_mul(tmp[:], src_get(i, ch), rbc[:])
                        eng.tensor_sub(d, tmp[:], mbc[:])
                    else:
                        eng.tensor_mul(d, src_get(i, ch), rbc[:])
                        eng.tensor_sub(d, d, mbc[:])
                if fp8_dst:
                    nc.scalar.copy(dst[:, NC_C, ch * 512:(ch + 1) * 512],
                                   dst[:, NC_C - 1, ch * 512:(ch + 1) * 512])

        def src_own(i, ch):
            return x_res[:, i, ch * 512:(ch + 1) * 512]

        def src_prt(i, ch):
            t_ = lnp.tile([P, 512], bf16, tag="ln_src")
            eng = (nc.sync, nc.gpsimd, nc.scalar)[(i + ch) % 3]
            eng.dma_start(out=t_[:], in_=xT_prt[i * P:(i + 1) * P,
                                               ch * 512:(ch + 1) * 512])
            return t_[:]

        def w_fm(w_dram, m, mode, nk=NC_C):
            """Load m-major weight tile [P, slots, 128] with a flat AP."""
            ns = n_wslots(mode, nk)
            wt = wp.tile([P, ns, P], wdt(mode), tag="w", name="wt")
            nc.sync.dma_start(
                out=wt[:].rearrange("p k j -> p (k j)"),
                in_=w_dram[:, m * ns * P:(m + 1) * ns * P])
            return wt

        def emit_contract(ps_ap, wt, rhs2, rhs3, mode, nk=NC_C):
            """Contraction over nk k-slabs into ps_ap (PSUM)."""
            if mode == 'bf16':
                for k in range(nk):
                    nc.tensor.matmul(ps_ap, wt[:, k, :], rhs2(k),
                                     start=(k == 0), stop=(k == nk - 1))
                return
            sched = dr_sched(mode, nk)
            for i, (ws, ks) in enumerate(sched):
                nc.tensor.matmul(ps_ap, wt[:, ws:ws + 2, :], rhs3(ks),
                                 start=(i == 0), stop=(i == len(sched) - 1),
                                 perf_mode=DR)

        def proj_fm(w_dram, mode, xsrc, m_tiles, evict_fn, n_tok=NTOK,
                    m_off=0, store=None):
            """Feature-major projection from x staging [P, XSLAB, n_tok].
            store=(dram, bias_t): batch all chunks of an m-tile into one
            [P, n_tok] buffer and emit a single DMA per m-tile."""
            nch = (n_tok + 511) // 512
            for m in range(m_tiles):
                wt = w_fm(w_dram, m_off + m, mode)
                st4 = (sp.tile([P, nch * 512], bf16, tag="st4", bufs=2,
                               name="st4") if store else None)
                for ch in range(nch):
                    c0 = ch * 512
                    cw = min(512, n_tok - c0)
                    ps = pp.tile([P, 512], f32, tag="ps")
                    rhs2 = lambda k: xsrc[:, k, c0:c0 + cw]
                    rhs3 = lambda k0: xsrc[:, k0:k0 + 2, c0:c0 + cw]
                    emit_contract(ps[:, :cw], wt, rhs2, rhs3, mode)
                    if store:
                        st_scaled(store[1], mode, m, ps, cw, m + ch,
                                  dst=st4[:, c0:c0 + cw])
                    else:
                        evict_fn(m, ch, ps, cw)
                if store:
                    eng = (nc.sync, nc.gpsimd, nc.scalar)[m % 3]
                    eng.dma_start(out=store[0][m * P:(m + 1) * P, :n_tok],
                                  in_=st4[:, :n_tok])

        def st_scaled(bias_t, mode, m, ps, cw, alt, dst=None):
            """PSUM -> bf16 SBUF with weight-scale fold + bias.
            Rotates Act / DVE so neither becomes the per-chunk bottleneck."""
            st = dst if dst is not None else sp.tile([P, 512], bf16, tag="st")
            if alt % 2 == 0:
                if bias_t is None:
                    nc.scalar.activation(st[:, :cw], ps[:, :cw], AF.Copy,
                                         scale=sc_of(mode))
                else:
                    nc.scalar.activation(st[:, :cw], ps[:, :cw], AF.Identity,
                                         bias=bias_t[:, m:m + 1],
                                         scale=sc_of(mode))
            else:
                if bias_t is None:
                    nc.vector.tensor_scalar_mul(out=st[:, :cw], in0=ps[:, :cw],
                                                scalar1=sc_of(mode))
                else:
                    nc.vector.scalar_tensor_tensor(
                        out=st[:, :cw], in0=ps[:, :cw], scalar=sc_of(mode),
                        in1=bias_t[:, m:m + 1].broadcast_to([P, cw]),
                        op0=ALU.mult, op1=ALU.add)
            return st

        def ev_plain(dram, bias_t, mode, m, ch, ps, cw):
            st = st_scaled(bias_t, mode, m, ps, cw, m + ch)
            (nc.scalar if (m + ch) % 2 else nc.sync).dma_start(
                out=dram[m * P:(m + 1) * P, ch * 512:ch * 512 + cw],
                in_=st[:, :cw])

        def proj_v(w_dram, mode, lhs_src, m_tiles, dst_row_of, mrows=P):
            """Token-major (V) projection: lhs_src [P, XSLAB, *]-like accessor
            giving per-slab [P, mrows] tiles; moving side is the weight.
            Output rows are [tok, NH*(HD+1)] with a gap col per head (later
            overwritten with ones for the fused PV+denominator matmul)."""
            ns = n_wslots(mode)
            for nch, (h0, nh) in enumerate(((0, 6), (6, 6), (12, 4))):
                c0, cw = h0 * HD, nh * HD
                wt = wp.tile([P, ns, 512], wdt(mode), tag="wbig", bufs=2)
                nc.sync.dma_start(
                    out=wt[:, :, :cw],
                    in_=w_dram.rearrange("p (k j) -> p k j", j=C)[:, :, c0:c0 + cw])
                for m in range(m_tiles):
                    ps = pp.tile([P, 512], f32, tag="ps")
                    if mode == 'bf16':
                        for k in range(NC_C):
                            nc.tensor.matmul(ps[:mrows, :cw], lhs_src(k, m),
                                             wt[:, k, :cw],
                                             start=(k == 0), stop=(k == NC_C - 1))
                    else:
                        sched = dr_sched(mode, NC_C)
                        for i, (ws, ks) in enumerate(sched):
                            nc.tensor.matmul(ps[:mrows, :cw],
                                             lhs_src((ks, ks + 2), m),
                                             wt[:, ws:ws + 2, :cw],
                                             start=(i == 0),
                                             stop=(i == len(sched) - 1),
                                             perf_mode=DR)
                    st = sp.tile([P, 512], bf16, tag="st")
                    if (m + nch) % 2:
                        nc.vector.tensor_scalar_mul(out=st[:mrows, :cw],
                                                    in0=ps[:mrows, :cw],
                                                    scalar1=sc_of(mode))
                    else:
                        nc.scalar.activation(st[:mrows, :cw], ps[:mrows, :cw],
                                             AF.Copy, scale=sc_of(mode))
                    dram, row0 = dst_row_of(m)
                    eng = (nc.sync, nc.gpsimd, nc.scalar)[(m + nch) % 3]
                    eng.dma_start(
                        out=dram[row0:row0 + mrows, :]
                            .rearrange("n (h d) -> n h d", d=HD + 1)
                            [:, h0:h0 + nh, :HD],
                        in_=st[:mrows, :cw].rearrange("p (h d) -> p h d",
                                                      d=HD))

        def lhs_of_x(xt, tok_of):
            """lhs accessor over x staging: k int -> [P, mrows]; (k0,k1) pair."""
            def get(k, m):
                t0, tn = tok_of(m)
                if isinstance(k, tuple):
                    return xt[:, k[0]:k[1], t0:t0 + tn]
                return xt[:, k, t0:t0 + tn]
            return get

        for rep in range(replicate):
            for i in range(NC_C):
                eng = (nc.sync, nc.gpsimd, nc.scalar)[i % 3]
                eng.dma_start(out=x_res[:, i, :], in_=xT_own[i * P:(i + 1) * P, :])

            # =================== LayerNorm + projection emission ===============
            x_ln = big.tile([P, XSLAB, NTOK], f8, tag="xact")
            layer_norm(src_own, x_ln)
            x_ln_prt = big.tile([P, XSLAB, NTOK], f8, tag="hT")

            # cross-attention K/V depend only on y — project them early and
            # keep results SBUF-resident for the cross phase.
            y_sb = cons.tile([P, XSLAB, P], f8, tag="y_sb")
            nc.sync.dma_start(out=y_sb[:].rearrange("p k j -> p (k j)"),
                              in_=yT8[:, :])
            proj_fm(wk_c, MODES['k_c'], y_sb, NC_C, None,
                    n_tok=YL, store=(kT_y, None))
            lhs_y = lhs_of_x(y_sb, lambda m: (0, YL))
            proj_v(wv_c, MODES['v_c'], lhs_y, 1, lambda m: (v_y, 0), mrows=YL)

            ky_sb = cons.tile([HD, NH, YL], bf16, tag="ky_sb")
            nc.sync.dma_start(out=ky_sb[:],
                              in_=kT_y.rearrange("(h j) n -> j h n", j=HD))
            vy2 = cons.tile([P, NH, HD + 1], bf16, tag="vy2")
            nc.sync.dma_start(
                out=vy2[:YL, :, :].rearrange("n h d -> n (h d)"),
                in_=v_y[:, :])
            nc.gpsimd.memset(vy2[:YL, :, HD:HD + 1], 1.0)

            # ---- spatial K own/prt -> kT_all cols [own|prt] per frame ----
            def ev_k(side):
                state = {}
                def ev(m, ch, ps, cw):
                    if ch == 0:
                        state['st4'] = sp.tile([P, 2048], bf16, tag="st4",
                                               bufs=2, name="st4")
                    st4 = state['st4']
                    st_scaled(None, MODES['k_s'], m, ps, cw, m + ch,
                              dst=st4[:, ch * 512:(ch + 1) * 512])
                    if ch == 3:
                        (nc.gpsimd if m % 2 else nc.sync).dma_start(
                            out=k_all_r[m * P:(m + 1) * P, :,
                                        side * SH:(side + 1) * SH],
                            in_=st4[:].rearrange("p (t s) -> p t s", s=SH))
                return ev
            proj_fm(wk_s, MODES['k_s'], x_ln, NC_C, ev_k(0))

            lhs_xln = lhs_of_x(x_ln, lambda m: (m * P, P))
            lhs_prt = lhs_of_x(x_ln_prt, lambda m: (m * P, P))

            proj_v(wv_s, MODES['v_s'], lhs_xln, NTT, lambda m: (v_s, m * S))
            proj_fm(wq_s, MODES['q_s'], x_ln, NC_C, None,
                    store=(qT_s, b_q_s))
            # partner-half LN issued AFTER own projections: its DVE work
            # overlaps the own-token matmul phase.
            layer_norm(src_prt, x_ln_prt)
            proj_fm(wk_s, MODES['k_s'], x_ln_prt, NC_C, ev_k(1))
            proj_v(wv_s, MODES['v_s'], lhs_prt, NTT, lambda m: (v_s, m * S + SH))

            # =================== attention ===================
            # Loads batched per 8 heads; softmax denominator rides as a
            # ones-column in the V tile so PV+denominator is one matmul.
            HB = 8                 # heads per load batch
            HD1 = HD + 1
            def attention(oT_dst, nk, q_of, k_of, v_of, masked, n_qb=NTT,
                          k_res=None, v_res=None):
                nkt = (nk + P - 1) // P
                fp8_out = oT_dst.dtype == f8
                for qb in range(n_qb):
                    o_acc = ap_.tile([P, C], bf16, tag="o_acc")
                    for hb in range(NH // HB):
                        if k_res is None:
                            kt8 = fr.tile([HD, HB, nkt * P], bf16, tag="kt",
                                          name="kt8")
                            (nc.scalar if (qb + hb) % 2 else nc.sync).dma_start(
                                out=kt8[:, :, :nk], in_=k_of(qb, hb))
                        qt8 = fr.tile([HD, HB, P], bf16, tag="qt", name="qt8")
                        (nc.gpsimd if (qb + hb) % 2 else nc.sync).dma_start(
                            out=qt8[:], in_=q_of(qb, hb))
                        if v_res is None:
                            vt8 = fr.tile([P, nkt, HB, HD1], bf16, tag="vt",
                                          name="vt8")
                            (nc.sync if qb % 2 else nc.gpsimd).dma_start(
                                out=vt8[:].rearrange("p k h d -> p k (h d)"),
                                in_=v_of(qb, hb))
                            nc.gpsimd.memset(vt8[:, :, :, HD:HD1], 1.0)
                        for w2 in range(HB // NW):
                            sc = [psc.tile([P, NW * P], f32, tag="sc",
                                           name="sc") for _ in range(nkt)]
                            for i in range(NW):
                                l = w2 * NW + i
                                for kt_i in range(nkt):
                                    kp = min(P, nk - kt_i * P)
                                    ktap = (kt8[:, l, kt_i * P:kt_i * P + kp]
                                            if k_res is None else
                                            k_res[:, hb * HB + l,
                                                  kt_i * P:kt_i * P + kp])
                                    nc.tensor.matmul(
                                        sc[kt_i][:kp, i * P:(i + 1) * P],
                                        ktap, qt8[:, l, :], start=(i == 0),
                                        stop=(i == NW - 1))
                            es = []
                            for kt_i in range(nkt):
                                kp = min(P, nk - kt_i * P)
                                e = ap_.tile([P, NW * P], bf16,
                                             tag="e%d" % kt_i, name="e")
                                if masked:
                                    nc.vector.scalar_tensor_tensor(
                                        out=sc[kt_i][:kp, :],
                                        in0=sc[kt_i][:kp, :],
                                        scalar=SCALE, in1=mask4[:kp, :, :]
                                        .rearrange("p w j -> p (w j)"),
                                        op0=ALU.mult, op1=ALU.add)
                                    nc.scalar.activation(e[:kp, :],
                                                         sc[kt_i][:kp, :],
                                                         AF.Exp)
                                else:
                                    nc.scalar.activation(e[:kp, :],
                                                         sc[kt_i][:kp, :],
                                                         AF.Exp, scale=SCALE)
                                es.append(e)
                            ov = pov.tile([P, 512], f32, tag="ov")
                            for i in range(NW):
                                l = w2 * NW + i
                                for kt_i in range(nkt):
                                    kp = min(P, nk - kt_i * P)
                                    vtap = (vt8[:kp, kt_i, l, :]
                                            if v_res is None else
                                            v_res[:kp, hb * HB + l, :])
                                    nc.tensor.matmul(
                                        ov[:, i * HD1:(i + 1) * HD1],
                                        es[kt_i][:kp, i * P:(i + 1) * P],
                                        vtap,
                                        start=(i == 0 and kt_i == 0),
                                        stop=(i == NW - 1 and kt_i == nkt - 1))
                            rec = ap_.tile([P, NW], f32, tag="rec")
                            nc.vector.reciprocal(
                                rec[:],
                                ov[:, :NW * HD1].rearrange(
                                    "p (w d) -> p w d", d=HD1)[:, :, HD])
                            h0 = (hb * HB + w2 * NW) * HD
                            nc.vector.tensor_mul(
                                o_acc[:, h0:h0 + NW * HD]
                                    .rearrange("p (w d) -> p w d", d=HD),
                                ov[:, :NW * HD1]
                                    .rearrange("p (w d) -> p w d",
                                               d=HD1)[:, :, :HD],
                                rec[:].rearrange("p (w o) -> p w o", o=1)
                                    .broadcast_to([P, NW, HD]))
                    for cb in range(NC_C):
                        tp = pov.tile([P, P], bf16, tag="ov", name="tp")
                        nc.tensor.transpose(tp[:], o_acc[:, cb * P:(cb + 1) * P],
                                            ident[:])
                        if cb % 2:
                            nc.scalar.copy(oT_dst[:, cb, qb * P:(qb + 1) * P], tp[:])
                        else:
                            nc.vector.tensor_copy(
                                out=oT_dst[:, cb, qb * P:(qb + 1) * P], in_=tp[:])
                        if fp8_out and cb == NC_C - 1:
                            nc.scalar.copy(oT_dst[:, NC_C, qb * P:(qb + 1) * P],
                                           tp[:])

            # ---- spatial attention ----
            q_s_r = qT_s.rearrange("(h j) (t s) -> j h t s", j=HD, t=T)
            k_sr = kT_all.rearrange("(h j) (t s) -> j h t s", j=HD, t=T)
            v_sr = v_s.rearrange("(t k p) c -> t p k c", k=2, p=P)
            HBW = HB * (HD + 1)
            oT_sp = big.tile([P, NC_C, NTOK], bf16, tag="xact")
            attention(
                oT_sp, S,
                q_of=lambda qb, hb: q_s_r[:, hb * HB:(hb + 1) * HB, qb, :],
                k_of=lambda qb, hb: k_sr[:, hb * HB:(hb + 1) * HB, qb, :],
                v_of=lambda qb, hb: v_sr[qb][:, :, hb * HBW:(hb + 1) * HBW],
                masked=False)

            # ---- residual projection (feature-major, adds into x_res) ----
            def proj_residual(w_dram, mode, xsrc, bias_t, scatter=False):
                for m in range(NC_C):
                    wt = w_fm(w_dram, m, mode)
                    for ch in range(NTOK // 512):
                        c0 = ch * 512
                        ps = pp.tile([P, 512], f32, tag="ps")
                        rhs2 = lambda k: xsrc[:, k, c0:c0 + 512]
                        rhs3 = lambda k0: xsrc[:, k0:k0 + 2, c0:c0 + 512]
                        emit_contract(ps[:], wt, rhs2, rhs3, mode)
                        st = st_scaled(bias_t, mode, m, ps, 512, m + ch)
                        eng = nc.vector if (m + ch) % 2 else nc.gpsimd
                        if not scatter:
                            eng.tensor_add(
                                out=x_res[:, m, c0:c0 + 512],
                                in0=x_res[:, m, c0:c0 + 512],
                                in1=st[:])
                        else:
                            # virtual chunk (g4,t,s) -> natural [t, s-slice]
                            xv = x_res[:, m, :] \
                                .rearrange("p (t s) -> p t s", t=T) \
                                [:, :, ch * 32:(ch + 1) * 32] \
                                .rearrange("p t (g s) -> p g t s", g=4)
                            eng.tensor_add(
                                out=xv,
                                in0=xv,
                                in1=st[:].rearrange("p (g t s) -> p g t s",
                                                    g=4, t=T))

            proj_residual(wproj_s, MODES['proj_s'], oT_sp, b_proj_s)

            # =================== temporal ===================
            # stage x_res + tpe into temporal-virtual token order (fp8);
            # one 4D-AP tensor op per feature slab
            x_virt = big.tile([P, XSLAB, NTOK], f8, tag="hT")
            for k in range(NC_C):
                xrk = x_res[:, k, :].rearrange("p (t g s) -> p g t s",
                                               t=T, g=NGRP)
                xvk = x_virt[:, k, :].rearrange("p (g t s) -> p g t s",
                                                g=NGRP, t=T)
                tk = tpev_sb[:, k, :].rearrange("p (g t s) -> p g t s",
                                                g=1, s=1)
                eng = nc.vector if k % 3 else nc.gpsimd
                eng.tensor_add(out=xvk, in0=xrk,
                               in1=tk.broadcast_to([P, NGRP, T, GRP]))
            for ch in range(NTOK // 512):
                nc.scalar.copy(x_virt[:, NC_C, ch * 512:(ch + 1) * 512],
                               x_virt[:, NC_C - 1, ch * 512:(ch + 1) * 512])

            proj_fm(wqk_t, MODES['qk_t'], x_virt, NC_C, None,
                    store=(qT_t, b_qk_t))
            proj_fm(wqk_t, MODES['qk_t'], x_virt, NC_C, None,
                    m_off=NC_C, store=(kT_t, None))
            lhs_virt = lhs_of_x(x_virt, lambda g: (g * P, P))
            proj_v(wv_t, MODES['v_t'], lhs_virt, NGRP, lambda g: (v_t, g * P))

            q_t_r = qT_t.rearrange("(h j) n -> j h n", j=HD)
            k_t_r = kT_t.rearrange("(h j) n -> j h n", j=HD)
            oT_t = big.tile([P, NC_C, NTOK], bf16, tag="xact")
            attention(
                oT_t, P,
                q_of=lambda qb, hb: q_t_r[:, hb * HB:(hb + 1) * HB,
                                          qb * P:(qb + 1) * P],
                k_of=lambda qb, hb: k_t_r[:, hb * HB:(hb + 1) * HB,
                                          qb * P:(qb + 1) * P],
                v_of=lambda qb, hb: v_t[qb * P:(qb + 1) * P,
                                        hb * HBW:(hb + 1) * HBW]
                    .rearrange("t (o hd) -> t o hd", o=1),
                masked=True, n_qb=NGRP)
            proj_residual(wproj_t, MODES['proj_t'], oT_t, b_proj_t, scatter=True)

            # =================== cross ===================
            # fp8 copy of x_res for the cross-Q projection (reuses xact buffer)
            x_res8 = big.tile([P, XSLAB, NTOK], f8, tag="xact")
            for m in range(NC_C):
                for ch in range(NTOK // 1024):
                    c0 = ch * 1024
                    eng = (nc.scalar, nc.vector, nc.gpsimd)[(m + ch) % 3]
                    if eng is nc.scalar:
                        eng.copy(x_res8[:, m, c0:c0 + 1024],
                                 x_res[:, m, c0:c0 + 1024])
                    else:
                        eng.tensor_copy(out=x_res8[:, m, c0:c0 + 1024],
                                        in_=x_res[:, m, c0:c0 + 1024])
            for ch in range(NTOK // 1024):
                nc.scalar.copy(x_res8[:, NC_C, ch * 1024:(ch + 1) * 1024],
                               x_res8[:, NC_C - 1, ch * 1024:(ch + 1) * 1024])

            proj_fm(wq_c, MODES['q_c'], x_res8, NC_C, None,
                    store=(qT_c, b_q_c))

            q_c_r = qT_c.rearrange("(h j) n -> j h n", j=HD)
            oT_c = big.tile([P, XSLAB, NTOK], f8, tag="hT")
            attention(
                oT_c, YL,
                q_of=lambda qb, hb: q_c_r[:, hb * HB:(hb + 1) * HB,
                                          qb * P:(qb + 1) * P],
                k_of=None, v_of=None,
                masked=False, k_res=ky_sb[:], v_res=vy2[:])
            proj_residual(wproj_c, MODES['proj_c'], oT_c, b_proj_c)

            # =================== MLP (bf16, token quarters) ===================
            x_ln2 = big.tile([P, NC_C, NTOK], bf16, tag="xact")
            layer_norm(src_own, x_ln2)

            NS2 = n_wslots(MODES['fc2'], 4 * NC_C)
            for quad in range(4):
                c0 = quad * 512
                hq = big.tile([P, 4 * NC_C, 512], bf16, tag="hT")
                # fc1 into SBUF (gelu), two m-tiles per weight load
                for m2 in range(2 * NC_C):
                    wt = wp.tile([P, 2, NC_C, P], wdt(MODES['fc1']),
                                 tag="wbig", bufs=2, name="wt")
                    nsl = n_wslots(MODES['fc1'])
                    nc.sync.dma_start(
                        out=wt[:].rearrange("p a k j -> p (a k j)"),
                        in_=wfc1[:, 2 * m2 * nsl * P:(2 * m2 + 2) * nsl * P])
                    for a in range(2):
                        m = 2 * m2 + a
                        ps = pp.tile([P, 512], f32, tag="ps")
                        rhs2 = lambda k: x_ln2[:, k, c0:c0 + 512]
                        rhs3 = lambda k0: x_ln2[:, k0:k0 + 2, c0:c0 + 512]
                        emit_contract(ps[:], wt[:, a], rhs2, rhs3,
                                      MODES['fc1'])
                        nc.scalar.activation(hq[:, m, :],
                                             ps[:], AF.Gelu_apprx_tanh,
                                             bias=b_fc1[:, m:m + 1],
                                             scale=sc_of(MODES['fc1']))
                # fc2 from SBUF
                for m in range(NC_C):
                    wt2 = wp.tile([P, NS2, P], wdt(MODES['fc2']), tag="wbig",
                                  bufs=2, name="wt2")
                    nc.sync.dma_start(
                        out=wt2[:].rearrange("p k j -> p (k j)"),
                        in_=wfc2[:, m * NS2 * P:(m + 1) * NS2 * P])
                    ps = pp.tile([P, 512], f32, tag="ps")
                    rhs2 = lambda k: hq[:, k, :]
                    rhs3 = lambda k0: hq[:, k0:k0 + 2, :]
                    emit_contract(ps[:], wt2, rhs2, rhs3, MODES['fc2'],
                                  nk=4 * NC_C)
                    st = st_scaled(b_fc2, MODES['fc2'], m, ps, 512, m + quad)
                    so = sp.tile([P, 512], f32, tag="st_out")
                    eng = nc.vector if (m + quad) % 2 else nc.gpsimd
                    eng.tensor_add(out=so[:], in0=st[:],
                                   in1=x_res[:, m, c0:c0 + 512])
                    (nc.sync if (m + quad) % 2 else nc.scalar).dma_start(
                        out=outT[m * P:(m + 1) * P, c0:c0 + 512],
                        in_=so[:])

    nc.finalize()
    return nc


# ======================= SPMD runner =======================
import time
import jax
from jax.sharding import Mesh, PartitionSpec
from jax.experimental.shard_map import shard_map
from concourse.bass2jax import _bass_exec_p, install_neuronx_cc_hook, partition_id_tensor

def make_runner(nc: bass.Bass, n_cores: int = 8):
    install_neuronx_cc_hook()
    assert nc.dbg_addr is None or not nc.dbg_callbacks

    partition_name = nc.partition_id_tensor.name if nc.partition_id_tensor else None
    in_names, out_names, out_avals, zero_outs = [], [], [], []
    for alloc in nc.m.functions[0].allocations:
        if not isinstance(alloc, mybir.MemoryLocationSet):
            continue
        name = alloc.memorylocations[0].name
        if alloc.kind == "ExternalInput":
            if name != partition_name:
                in_names.append(name)
        elif alloc.kind == "ExternalOutput":
            out_names.append(name)
            shape = tuple(alloc.tensor_shape)
            dtype = mybir.dt.np(alloc.dtype)
            out_avals.append(jax.core.ShapedArray(shape, dtype))
            zero_outs.append(np.zeros(shape, dtype))
    n_params = len(in_names)
    n_outs = len(out_avals)
    all_in_names = list(in_names) + list(out_names)
    if partition_name is not None:
        all_in_names.append(partition_name)

    def _body(*args):
        operands = list(args)
        if partition_name is not None:
            operands.append(partition_id_tensor())
        outs = _bass_exec_p.bind(
            *operands,
            out_avals=tuple(out_avals),
            in_names=tuple(all_in_names),
            out_names=tuple(out_names),
            lowering_input_output_aliases=(),
            sim_require_finite=True,
            sim_require_nnan=True,
            nc=nc,
        )
        return tuple(outs)

    devices = jax.devices()[:n_cores]
    mesh = Mesh(np.asarray(devices), ("core",))
    in_specs = (PartitionSpec("core"),) * (n_params + n_outs)
    out_specs = (PartitionSpec("core"),) * n_outs
    donate = tuple(range(n_params, n_params + n_outs))
    sharded = jax.jit(
        shard_map(_body, mesh=mesh, in_specs=in_specs, out_specs=out_specs,
                  check_rep=False),
        donate_argnums=donate, keep_unused=True,
    )

    sharding = jax.sharding.NamedSharding(mesh, PartitionSpec("core"))

    def run(in_maps, n_iters=3):
        per_core = [[np.asarray(m[name]) for name in in_names] for m in in_maps]
        concat_in = [
            np.concatenate([per_core[c][i] for c in range(n_cores)], axis=0)
            for i in range(n_params)
        ]
        dev_in = [jax.device_put(a, sharding) for a in concat_in]
        times = []
        out_arrs = None
        for it in range(n_iters):
            dev_zeros = [
                jax.device_put(np.zeros((n_cores * z.shape[0], *z.shape[1:]), z.dtype),
                               sharding)
                for z in zero_outs
            ]
            for z in dev_zeros:
                z.block_until_ready()
            t0 = time.perf_counter()
            out = sharded(*dev_in, *dev_zeros)
            for o in out:
                o.block_until_ready()
            t1 = time.perf_counter()
            times.append(t1 - t0)
            out_arrs = out
        results = [
            {
                name: np.asarray(out_arrs[i]).reshape(n_cores, *out_avals[i].shape)[c]
                for i, name in enumerate(out_names)
            }
            for c in range(n_cores)
        ]
        return results, times

    return run


# ======================= host prep + entry point =======================
import ml_dtypes

B = 4
bfloat16 = ml_dtypes.bfloat16
F8 = ml_dtypes.float8_e4m3
F8MAX = 240.0


def _bf(x):
    return np.ascontiguousarray(x, dtype=np.float32).astype(bfloat16)


def _q8(x):
    return np.clip(np.asarray(x, np.float32) * WS, -F8MAX, F8MAX).astype(F8)


def _q8lo(x):
    s = np.asarray(x, np.float32) * WS
    hi = np.clip(s, -F8MAX, F8MAX).astype(F8)
    lo = np.clip(s - hi.astype(np.float32), -F8MAX, F8MAX).astype(F8)
    return hi, lo


def _slabs(wT):
    """W_T (Cin, M) -> [nk, P, M] f32 slab list."""
    Cin, M = wT.shape
    nk = Cin // P
    return np.asarray(wT, np.float32).reshape(nk, P, M)


def _quant_slots(sl, mode):
    """sl [nk, P, M] -> [S, P, M] quantized slot stack per mode."""
    nk = sl.shape[0]
    if mode == 'bf16':
        return sl.astype(bfloat16)
    if mode == 'fp8':
        if nk % 2:
            half = sl[nk - 1] * 0.5
            return np.stack([_q8(sl[k]) for k in range(nk - 1)]
                            + [_q8(half), _q8(half)], 0)
        return np.stack([_q8(sl[k]) for k in range(nk)], 0)
    if mode == 'wcomp':
        his, los = zip(*[_q8lo(sl[k]) for k in range(nk)])
        if nk % 2:
            order = list(his[:nk - 1]) + [his[nk - 1], los[nk - 1]] \
                + list(los[:nk - 1])
        else:
            order = list(his) + list(los)
        return np.stack(order, 0)
    raise ValueError(mode)


def _fm(wT, mode):
    """W_T (Cin, M) -> m-major [p, m, slot, j] flat device layout."""
    Cin, M = wT.shape
    nm = M // P
    q = _quant_slots(_slabs(wT), mode)            # [S, P, M]
    ns = q.shape[0]
    w = q.reshape(ns, P, nm, P).transpose(1, 2, 0, 3).reshape(P, nm * ns * P)
    return np.ascontiguousarray(w)


def _jc(wT, mode):
    """W_T (Cin, M) -> j-contiguous [p, slot, M] device layout."""
    q = _quant_slots(_slabs(wT), mode)            # [S, P, M]
    ns, _, M = q.shape
    return np.ascontiguousarray(q.transpose(1, 0, 2).reshape(P, ns * M))


def build_in_maps(inputs):
    x = np.asarray(inputs['x'], np.float32)
    y = np.asarray(inputs['y'], np.float32)
    t = np.asarray(inputs['t'], np.float32)
    tpe = np.asarray(inputs['tpe'], np.float32)
    sst = np.asarray(inputs['sst'], np.float32)
    W = {k: np.asarray(inputs[k], np.float32) for k in inputs
         if k not in ('x', 'y', 't', 'tpe', 'sst')}

    t6 = sst[None] + t.reshape(B, 6, C)
    sh_msa, sc_msa, g_msa, sh_mlp, sc_mlp, g_mlp = [t6[:, i] for i in range(6)]

    mask = np.zeros((P, P), np.float32)
    t2 = np.arange(P)[:, None] // GRP
    s2 = np.arange(P)[:, None] % GRP
    t1 = np.arange(P)[None, :] // GRP
    s1 = np.arange(P)[None, :] % GRP
    mask[~((s2 == s1) & (t2 <= t1))] = -30000.0

    # tpe feature-major [p, k, t]
    tpe_fm = tpe[0].T.reshape(NC_C, P, T).transpose(1, 0, 2).reshape(P, NC_C * T)

    in_maps = []
    for b in range(B):
        wqkv_s = W['qkv_s_w'] * (1.0 + sc_msa[b])[None, :]
        bqkv_s = W['qkv_s_w'] @ sh_msa[b] + W['qkv_s_b']
        wproj_s = W['proj_s_w'] * g_msa[b][:, None]
        bproj_s = g_msa[b] * W['proj_s_b'] + wproj_s @ bqkv_s[2 * C:]
        wproj_t = W['proj_t_w'] * g_msa[b][:, None]
        bproj_t = (g_msa[b] * W['proj_t_b']
                   + wproj_t @ W['qkv_t_b'][2 * C:])
        bv_c = W['kv_c_b'][C:]
        bproj_c = W['proj_c_b'] + W['proj_c_w'] @ bv_c
        wfc1 = W['fc1_w'] * (1.0 + sc_mlp[b])[None, :]
        bfc1 = W['fc1_w'] @ sh_mlp[b] + W['fc1_b']
        wfc2 = W['fc2_w'] * g_mlp[b][:, None]
        bfc2 = g_mlp[b] * W['fc2_b']

        # y staging: [p, slot, YL] fp8, slab 8 duplicated at slot 9
        yT = y[b].T                                  # (C, YL)
        ysl = yT.reshape(NC_C, P, YL)
        y8 = np.clip(ysl, -F8MAX, F8MAX).astype(F8)
        y8 = np.concatenate([y8, y8[NC_C - 1:NC_C]], 0)      # [10, P, YL]
        y8p = np.zeros((XSLAB, P, P), F8)
        y8p[:, :, :YL] = y8
        y8 = np.ascontiguousarray(
            y8p.transpose(1, 0, 2).reshape(P, XSLAB * P))

        common = dict(
            yT8=y8,
            wq_s=_fm(wqkv_s[:C].T, MODES['q_s']),
            wk_s=_fm(wqkv_s[C:2 * C].T, MODES['k_s']),
            wv_s=_jc(wqkv_s[2 * C:].T, MODES['v_s']),
            wproj_s=_fm(wproj_s.T, MODES['proj_s']),
            wqk_t=np.concatenate(
                [_fm(W['qkv_t_w'][:C].T, MODES['qk_t']),
                 _fm(W['qkv_t_w'][C:2 * C].T, MODES['qk_t'])], axis=1),
            wv_t=_jc(W['qkv_t_w'][2 * C:].T, MODES['v_t']),
            wproj_t=_fm(wproj_t.T, MODES['proj_t']),
            wq_c=_fm(W['q_c_w'].T, MODES['q_c']),
            wk_c=_fm(W['kv_c_w'][:C].T, MODES['k_c']),
            wv_c=_jc(W['kv_c_w'][C:].T, MODES['v_c']),
            wproj_c=_fm(W['proj_c_w'].T, MODES['proj_c']),
            wfc1=_fm(wfc1.T, MODES['fc1']),
            wfc2=_fm(wfc2.T, MODES['fc2']),
            bq_s=np.ascontiguousarray(bqkv_s[:C], np.float32),
            bqk_t=np.ascontiguousarray(W['qkv_t_b'][:2 * C], np.float32),
            bq_c=np.ascontiguousarray(W['q_c_b'], np.float32),
            bproj_s=np.ascontiguousarray(bproj_s, np.float32),
            bproj_t=np.ascontiguousarray(bproj_t, np.float32),
            bproj_c=np.ascontiguousarray(bproj_c, np.float32),
            bfc1=np.ascontiguousarray(bfc1, np.float32),
            bfc2=np.ascontiguousarray(bfc2, np.float32),
            tpev=_bf(tpe_fm),
            maskneg=_bf(mask),
        )
        xb = x[b].reshape(T, S, C)
        for sh in range(2):
            own = xb[:, sh * SH:(sh + 1) * SH, :].reshape(NTOK, C)
            prt = xb[:, (1 - sh) * SH:(2 - sh) * SH, :].reshape(NTOK, C)
            m = dict(common)
            m['xT_own'] = _bf(own.T)
            m['xT_prt'] = _bf(prt.T)
            in_maps.append(m)
    return in_maps


def assemble(outs):
    xout = np.zeros((B, T * S, C), np.float32)
    ci = 0
    for b in range(B):
        for sh in range(2):
            o = outs[ci]['outT']            # (C, NTOK)
            tok = o.T.reshape(T, SH, C)
            xout[b].reshape(T, S, C)[:, sh * SH:(sh + 1) * SH, :] = tok
            ci += 1
    return xout


_CACHE = {}


def run_kernel(inputs, replicate=1, n_iters=2):
    key = replicate
    if key not in _CACHE:
        nc = build(replicate)
        _CACHE[key] = make_runner(nc, 8)
    run = _CACHE[key]
    in_maps = build_in_maps(inputs)
    results, times = run(in_maps, n_iters=n_iters)
    return assemble(results), times


def kernel(**inputs):
    out, _ = run_kernel(inputs, replicate=1, n_iters=1)
    return out
